# revision 22
# baseline (speedup 1.0000x reference)
"""BertSelfAttention on 8 Trainium2 NeuronCores.

Sharding: data parallel over batch (B=2) x tensor parallel over heads
(16 heads -> 4 groups of 4). Core c handles batch c//4, heads 4*(c%4)..+4.
No collectives: each core produces a disjoint [256, 2048] output slice
(feature-major); the host transposes/concatenates.

Per-core program (identical on all cores, SPMD over data):
  inputs (host-prepped):
    xt    [1024, 2048]  hidden_states[b].T                     (bf16)
    wq/wk/wv [1024, 256] weight column slices, bf16 (wq,qb2 pre-scaled 1/8)
    qb2/kb2 [128, 2]    bias chunks (per-partition layout, f32)
    vb    [1, 256]      bf16
    em    [128, 16]     exp(mask) chunks (f32)   em4 [128, 64] bf16 (x4 rep)
  output:
    out   [256, 2048]   context slice, feature-major (f32)

The attention mask is folded into V: softmax(s+m) @ V == (exp(s) @
diag(exp(m)) V_aug) / (exp(s) @ diag(exp(m)) ones), so V rows (and the
denominator ones-column) are pre-scaled by exp(mask) and the exp on the
Activation engine needs no per-partition bias operand.

Pipeline: one continuous phase. Per (pair p, query-block sp): 16 S steps
(2 matmuls each: 1 key tile x 2 heads, bf16, d=64 contraction) each
followed by one exp ([128, 1024], double-buffered PSUM so S(kt+1) never
waits on exp(kt)); between S steps the PE stream is filled with
V-projection / pair-1 projection / previous block's context units. Context: V_aug-tile.T @ expS
accumulated over 16 key tiles (M=65: rows 0-63 ctx.T, row 64 denom);
normalize = DVE reciprocal_approx_fast + GPSIMD partition_broadcast +
DVE multiply, then DMA out.
"""

import numpy as np

HIDDEN = 1024
HEADS = 16
HD = 64
B = 2
S = 2048
NCORES = 8
HPC = HEADS // 4  # heads per core = 4
WCOLS = HPC * HD  # 256 weight columns per core

_CACHE = {}


def _build_program():
    import concourse.bass as bass
    import concourse.bacc as bacc
    import concourse.tile as tile
    import concourse.mybir as mybir

    f32 = mybir.dt.float32
    bf16 = mybir.dt.bfloat16
    Exp = mybir.ActivationFunctionType.Exp

    nc = bacc.Bacc("TRN2", target_bir_lowering=False, debug=False, num_devices=NCORES)

    xt_d = nc.dram_tensor("xt", [HIDDEN, S], bf16, kind="ExternalInput")
    wq_d = nc.dram_tensor("wq", [HIDDEN, WCOLS], bf16, kind="ExternalInput")
    wk_d = nc.dram_tensor("wk", [HIDDEN, WCOLS], bf16, kind="ExternalInput")
    wv_d = nc.dram_tensor("wv", [HIDDEN, WCOLS], bf16, kind="ExternalInput")
    qb_d = nc.dram_tensor("qb2", [128, 2], f32, kind="ExternalInput")
    kb_d = nc.dram_tensor("kb2", [128, 2], f32, kind="ExternalInput")
    em_d = nc.dram_tensor("em", [128, 16], f32, kind="ExternalInput")
    em4_d = nc.dram_tensor("em4", [128, 64], bf16, kind="ExternalInput")
    out_d = nc.dram_tensor("out", [WCOLS, S], f32, kind="ExternalOutput")

    xt_r = xt_d.ap().rearrange("(c p) m -> p c m", p=128)  # [128, 8, 2048]
    wq_r = wq_d.ap().rearrange("(c p) n -> p c n", p=128)  # [128, 8, 256]
    wk_r = wk_d.ap().rearrange("(c p) n -> p c n", p=128)
    wv_r = wv_d.ap().rearrange("(c p) n -> p c n", p=128)

    with (
        tile.TileContext(nc) as tc,
        tc.tile_pool(name="main", bufs=1) as P,
        tc.tile_pool(name="att", bufs=2) as att,
        tc.tile_pool(name="nrm", bufs=2) as nrm,
        tc.tile_pool(name="pqp", bufs=2, space="PSUM") as PQ,
        tc.tile_pool(name="psp", bufs=2, space="PSUM") as PS,
        tc.tile_pool(name="pcp", bufs=2, space="PSUM") as PC,
    ):
        xt = [P.tile([128, S], bf16, name=f"xt{k}") for k in range(8)]
        wq_sb = P.tile([128, 8, WCOLS], bf16)
        wk_sb = P.tile([128, 8, WCOLS], bf16)
        wv_sb = P.tile([128, 8, WCOLS], bf16)
        q_sb = P.tile([128, 2, S], bf16)  # [feat(2 heads), pair, token]
        k_sb = P.tile([128, 2, S], bf16)
        v_sb = P.tile([128, 16, 4 * 65], bf16)  # [token, tile, 4*(em + 64 feats)]
        v_blk = v_sb.rearrange("p m (l c) -> p m l c", l=4)
        qkb = P.tile([128, 4], f32)
        qb_sb = qkb[:, 0:2]
        kb_sb = qkb[:, 2:4]
        em_sb = P.tile([128, 16], f32)
        em4_sb = P.tile([128, 64], bf16)

        # ---- input DMAs on two parallel queues (sync + gpsimd; the ACT
        # queue must stay clear for the exp stream). First halves of the
        # hidden-state chunks land first: they cover query/key blocks
        # sp0/sp1, which is all the prologue and the first S steps need.
        nc.sync.dma_start(out=wq_sb[:, :, 0:128], in_=wq_r[:, :, 0:128])
        nc.gpsimd.dma_start(out=qb_sb, in_=qb_d.ap())
        nc.gpsimd.dma_start(out=kb_sb, in_=kb_d.ap())
        nc.gpsimd.dma_start(out=wk_sb[:, :, 0:128], in_=wk_r[:, :, 0:128])
        nc.sync.dma_start(out=wq_sb[:, :, 128:256], in_=wq_r[:, :, 128:256])
        nc.gpsimd.dma_start(out=wk_sb[:, :, 128:256], in_=wk_r[:, :, 128:256])
        h0 = slice(0, 1024)
        h1 = slice(1024, 2048)
        for k in range(4):
            nc.sync.dma_start(out=xt[k][:, h0], in_=xt_r[:, k, h0])
        for k in range(4, 8):
            nc.gpsimd.dma_start(out=xt[k][:, h0], in_=xt_r[:, k, h0])
        nc.gpsimd.dma_start(out=wv_sb[:], in_=wv_r)
        nc.gpsimd.dma_start(out=em_sb[:], in_=em_d.ap())
        nc.gpsimd.dma_start(out=em4_sb[:], in_=em4_d.ap())
        for k in range(4):
            nc.sync.dma_start(out=xt[k][:, h1], in_=xt_r[:, k, h1])
        for k in range(4, 8):
            nc.gpsimd.dma_start(out=xt[k][:, h1], in_=xt_r[:, k, h1])

        # ---- unit generators ----
        # contraction chunks in DMA-arrival order (chunks alternate between
        # the sync and gpsimd queues), so the prologue is never stuck
        # waiting for chunk k while k+1 already landed
        KORD = [0, 4, 1, 5, 2, 6, 3, 7]

        def proj_unit(w_sb, b_sb, dst, mc, sp):
            # one [128 feats, 512 tokens] projection block: 8 matmuls + bias
            pq = PQ.tile([128, 512], f32, tag="pq")
            for i, k in enumerate(KORD):
                nc.tensor.matmul(
                    pq[:],
                    lhsT=w_sb[:, k, mc * 128 : mc * 128 + 128],
                    rhs=xt[k][:, sp * 512 : sp * 512 + 512],
                    start=(i == 0),
                    stop=(i == 7),
                )
            nc.vector.tensor_scalar_add(
                dst[:, mc, sp * 512 : sp * 512 + 512], pq[:], b_sb[:, mc : mc + 1]
            )

        def v_unit(mt):
            # V token tile [128 tokens, 256 feats] + bias row, scaled by exp(mask)
            pv = PQ.tile([128, 512], f32, tag="pq", name="pv")[:, 0:256]
            for i, k in enumerate(KORD):
                nc.tensor.matmul(
                    pv[:],
                    lhsT=xt[k][:, mt * 128 : mt * 128 + 128],
                    rhs=wv_sb[:, k, :],
                    start=(i == 0),
                    stop=(i == 7),
                )
            nc.vector.tensor_scalar_mul(
                v_blk[:, mt, :, 1:65],
                pv.rearrange("p (l c) -> p l c", l=4),
                em_sb[:, mt : mt + 1],
            )
            nc.vector.tensor_copy(v_blk[:, mt, :, 0], em4_sb[:, 4 * mt : 4 * mt + 4])

        def s_kt(p, sp, kt, expP):
            # S^T for key tile kt (both heads of pair p) + mask-free exp
            qs = sp * 512
            ps = PS.tile([128, 1024], f32, tag="ps")
            for h in range(2):
                rs = 64 * h
                nc.tensor.matmul(
                    ps[:, h * 512 : h * 512 + 512],
                    lhsT=k_sb[rs : rs + 64, p, kt * 128 : kt * 128 + 128],
                    rhs=q_sb[rs : rs + 64, p, qs : qs + 512],
                    start=True,
                    stop=True,
                )
            nc.scalar.activation(expP[:, kt, :], ps[:], Exp)

        pcs = {}

        def c_unit(p, sp, half, u, expP):
            # 4 accumulation steps of ctx^T; u==3 finishes with normalize+DMA
            lh = 2 * p + half
            key = (p, sp, half)
            if u == 0:
                pcs[key] = PC.tile([65, 512], f32, tag="pc", name=f"pc{lh}")
            pc = pcs[key]
            for j in range(4):
                kt = 4 * u + j
                nc.tensor.matmul(
                    pc[:],
                    lhsT=v_sb[:, kt, 65 * lh : 65 * lh + 65],
                    rhs=expP[:, kt, half * 512 : half * 512 + 512],
                    start=(kt == 0),
                    stop=(kt == 15),
                )
            if u == 3:
                qs = sp * 512
                ctxs = nrm.tile([65, 512], f32, tag="ctxs")
                bc = nrm.tile([65, 512], f32, tag="bc")
                nq = 2 if (p, sp) == (1, 3) else 1
                for q in range(nq):
                    cs = slice(q * 512 // nq, (q + 1) * 512 // nq)
                    nc.vector.reciprocal_approx_fast(ctxs[0:1, cs], pc[0:1, cs])
                    nc.gpsimd.partition_broadcast(bc[:, cs], ctxs[0:1, cs])
                    nc.vector.tensor_mul(ctxs[:, cs], pc[:, cs], bc[:, cs])
                    nc.sync.dma_start(
                        out=out_d.ap()[
                            64 * lh : 64 * lh + 64, qs + cs.start : qs + cs.stop
                        ],
                        in_=ctxs[1:65, cs],
                    )

        # ---- schedule ----
        # prologue: just Q(0,0) + K(0,s0/s1) -- S(0,0,kt) only needs the
        # key tiles up to kt, so K(0,s2/s3) ride as the first fillers of
        # block (0,0) (they complete before S reaches key tile 8)
        proj_unit(wq_sb, qb_sb, q_sb, 0, 0)
        proj_unit(wk_sb, kb_sb, k_sb, 0, 0)

        blocks = [(p, sp) for p in (0, 1) for sp in range(4)]
        expPs = {}

        # filler lists per block: context units of the previous block plus
        # projection units needed by upcoming S blocks (emitted late in the
        # block, always before their consumer's S-group in program order).
        def pq_unit(p, sp):
            return ("proj", wq_sb, qb_sb, q_sb, p, sp)

        def pk_unit(p, sp):
            return ("proj", wk_sb, kb_sb, k_sb, p, sp)

        fillers = {
            (0, 0): [pk_unit(0, 1)]
                    + [("v", mt) for mt in range(8)]
                    + [pk_unit(0, 2), pk_unit(0, 3)]
                    + [("v", mt) for mt in range(8, 16)] + [pq_unit(0, 1)],
            (0, 1): [("c", 0, 0)] + [pq_unit(0, 2)],
            (0, 2): [("c", 0, 1)] + [pq_unit(0, 3)],
            (0, 3): [("c", 0, 2), pk_unit(1, 0), pk_unit(1, 1), pk_unit(1, 2),
                     pk_unit(1, 3), pq_unit(1, 0)],
            (1, 0): [("c", 0, 3), pq_unit(1, 1)],
            (1, 1): [("c", 1, 0), pq_unit(1, 2)],
            (1, 2): [("c", 1, 1), pq_unit(1, 3)],
            (1, 3): [("c", 1, 2)],
        }

        COST = {"v": 1.0, "proj": 1.8, "cu": 0.85}

        def expand(fill):
            out = []
            for f in fill:
                if f[0] == "c":
                    _, cp, csp = f
                    for u in range(4):
                        for half in range(2):
                            out.append(("cu", cp, csp, half, u))
                else:
                    out.append(f)
            return out

        def emit(unit):
            if unit[0] == "v":
                v_unit(unit[1])
            elif unit[0] == "proj":
                _, w, b, dst, mc, sp = unit
                proj_unit(w, b, dst, mc, sp)
            elif unit[0] == "cu":
                _, cp, csp, half, u = unit
                c_unit(cp, csp, half, u, expPs[(cp, csp)])

        for p, sp in blocks:
            last = (p, sp) == (1, 3)
            expP = att.tile([128, 16, 1024], bf16, tag="expP")
            expPs[(p, sp)] = expP
            fill = expand(fillers[(p, sp)])
            total = sum(COST[f[0]] for f in fill) or 1.0
            done = 0.0
            idx = 0
            for kt in range(16):
                s_kt(p, sp, kt, expP)
                if last:
                    # front-load this block's fillers, then chase the exps
                    # with our own context units at a one-key-tile lag
                    budget = 2 if kt < 8 else 0
                    while budget > 0 and idx < len(fill):
                        emit(fill[idx])
                        idx += 1
                        budget -= 1
                    if kt in (5, 9, 13):
                        u = (kt - 5) // 4
                        c_unit(1, 3, 0, u, expP)
                        c_unit(1, 3, 1, u, expP)
                else:
                    target = total * (kt + 1) / 16.0
                    while idx < len(fill) and done < target - 1e-9:
                        emit(fill[idx])
                        done += COST[fill[idx][0]]
                        idx += 1
            while idx < len(fill):
                emit(fill[idx])
                idx += 1

        # epilogue: the last context chunk (its exps just finished)
        for half in range(2):
            c_unit(1, 3, half, 3, expPs[(1, 3)])

    nc.compile()
    return nc


def _get_program():
    if "nc" not in _CACHE:
        _CACHE["nc"] = _build_program()
    return _CACHE["nc"]


def _to_bf16(x):
    import ml_dtypes

    return np.asarray(x, np.float32).astype(ml_dtypes.bfloat16)


def _make_in_maps(hidden_states, attention_mask, q_w, q_b, k_w, k_b, v_w, v_b):
    hs = np.asarray(hidden_states, np.float32)
    am = np.asarray(attention_mask, np.float32)
    q_w = np.asarray(q_w, np.float32)
    k_w = np.asarray(k_w, np.float32)
    v_w = np.asarray(v_w, np.float32)
    q_b = np.asarray(q_b, np.float32)
    k_b = np.asarray(k_b, np.float32)
    v_b = np.asarray(v_b, np.float32)

    scale = np.float32(1.0 / np.sqrt(HD))

    in_maps = []
    for c in range(NCORES):
        b = c // 4
        hg = c % 4
        cols = slice(WCOLS * hg, WCOLS * hg + WCOLS)
        mask = am[b, 0, 0, :]  # [S]
        em = np.exp(mask.reshape(16, 128).T).astype(np.float32)  # [128, 16]
        em4 = np.repeat(em[:, :, None], 4, axis=2).reshape(128, 64)
        in_maps.append(
            {
                "xt": np.ascontiguousarray(_to_bf16(hs[b].T)),
                "wq": np.ascontiguousarray(_to_bf16(q_w[:, cols] * scale)),
                "wk": np.ascontiguousarray(_to_bf16(k_w[:, cols])),
                "wv": np.ascontiguousarray(_to_bf16(v_w[:, cols])),
                "qb2": np.ascontiguousarray((q_b[cols] * scale).reshape(2, 128).T),
                "kb2": np.ascontiguousarray(k_b[cols].reshape(2, 128).T),
                "em": np.ascontiguousarray(em),
                "em4": np.ascontiguousarray(_to_bf16(em4)),
            }
        )
    return in_maps


def kernel(hidden_states, attention_mask, q_w, q_b, k_w, k_b, v_w, v_b):
    from concourse import bass_utils

    nc = _get_program()
    in_maps = _make_in_maps(
        hidden_states, attention_mask, q_w, q_b, k_w, k_b, v_w, v_b
    )
    _CACHE["in_maps"] = in_maps
    res = bass_utils.run_bass_kernel_spmd(nc, in_maps, core_ids=list(range(NCORES)))

    full = np.empty((B, S, HIDDEN), np.float32)
    for c in range(NCORES):
        b = c // 4
        hg = c % 4
        full[b, :, WCOLS * hg : WCOLS * hg + WCOLS] = res.results[c]["out"].T
    # V bias contributes exactly v_b to every context vector (softmax
    # weights sum to 1), so it is added here instead of on-device.
    full += np.asarray(v_b, np.float32)[None, None, :]
    return full


# revision 23
# speedup vs baseline: 1.1820x; 1.1820x over previous
"""BertSelfAttention on 8 Trainium2 NeuronCores.

Sharding: data parallel over batch (B=2) x tensor parallel over heads
(16 heads -> 4 groups of 4). Core c handles batch c//4, heads 4*(c%4)..+4.
No collectives: each core produces a disjoint [256, 2048] output slice
(feature-major); the host transposes/concatenates.

Per-core program (identical on all cores, SPMD over data):
  inputs (host-prepped):
    xt    [1024, 2048]  hidden_states[b].T                     (bf16)
    wq/wk/wv [1024, 256] weight column slices, bf16 (wq,qb2 pre-scaled 1/8)
    qb2/kb2 [128, 2]    bias chunks (per-partition layout, f32)
    vb    [1, 256]      bf16
    em    [128, 16]     exp(mask) chunks (f32)   em4 [128, 64] bf16 (x4 rep)
  output:
    out   [256, 2048]   context slice, feature-major (f32)

The attention mask is folded into V: softmax(s+m) @ V == (exp(s) @
diag(exp(m)) V_aug) / (exp(s) @ diag(exp(m)) ones), so V rows (and the
denominator ones-column) are pre-scaled by exp(mask) and the exp on the
Activation engine needs no per-partition bias operand.

Pipeline: one continuous phase. Per (pair p, query-block sp): 16 S steps
(2 matmuls each: 1 key tile x 2 heads, bf16, d=64 contraction) each
followed by one exp ([128, 1024], double-buffered PSUM so S(kt+1) never
waits on exp(kt)); between S steps the PE stream is filled with
V-projection / pair-1 projection / previous block's context units. Context: V_aug-tile.T @ expS
accumulated over 16 key tiles (M=65: rows 0-63 ctx.T, row 64 denom);
normalize = DVE reciprocal_approx_fast + GPSIMD partition_broadcast +
DVE multiply, then DMA out.
"""

import numpy as np

HIDDEN = 1024
HEADS = 16
HD = 64
B = 2
S = 2048
NCORES = 8
HPC = HEADS // 4  # heads per core = 4
WCOLS = HPC * HD  # 256 weight columns per core

_CACHE = {}


def _build_program():
    import concourse.bass as bass
    import concourse.bacc as bacc
    import concourse.tile as tile
    import concourse.mybir as mybir

    f32 = mybir.dt.float32
    bf16 = mybir.dt.bfloat16
    Exp = mybir.ActivationFunctionType.Exp

    nc = bacc.Bacc("TRN2", target_bir_lowering=False, debug=False, num_devices=NCORES)

    xt_d = nc.dram_tensor("xt", [HIDDEN, S], bf16, kind="ExternalInput")
    wq_d = nc.dram_tensor("wq", [HIDDEN, WCOLS], bf16, kind="ExternalInput")
    wk_d = nc.dram_tensor("wk", [HIDDEN, WCOLS], bf16, kind="ExternalInput")
    wv_d = nc.dram_tensor("wv", [HIDDEN, WCOLS], bf16, kind="ExternalInput")
    qb_d = nc.dram_tensor("qb2", [128, 2], f32, kind="ExternalInput")
    kb_d = nc.dram_tensor("kb2", [128, 2], f32, kind="ExternalInput")
    em_d = nc.dram_tensor("em", [128, 16], f32, kind="ExternalInput")
    em4_d = nc.dram_tensor("em4", [128, 64], bf16, kind="ExternalInput")
    out_d = nc.dram_tensor("out", [WCOLS, S], f32, kind="ExternalOutput")

    xt_r = xt_d.ap().rearrange("(c p) m -> p c m", p=128)  # [128, 8, 2048]
    wq_r = wq_d.ap().rearrange("(c p) n -> p c n", p=128)  # [128, 8, 256]
    wk_r = wk_d.ap().rearrange("(c p) n -> p c n", p=128)
    wv_r = wv_d.ap().rearrange("(c p) n -> p c n", p=128)

    with (
        tile.TileContext(nc) as tc,
        tc.tile_pool(name="main", bufs=1) as P,
        tc.tile_pool(name="att", bufs=2) as att,
        tc.tile_pool(name="nrm", bufs=2) as nrm,
        tc.tile_pool(name="pqp", bufs=2, space="PSUM") as PQ,
        tc.tile_pool(name="psp", bufs=2, space="PSUM") as PS,
        tc.tile_pool(name="pcp", bufs=2, space="PSUM") as PC,
    ):
        xt = [P.tile([128, S], bf16, name=f"xt{k}") for k in range(8)]
        wq_sb = P.tile([128, 8, WCOLS], bf16)
        wk_sb = P.tile([128, 8, WCOLS], bf16)
        wv_sb = P.tile([128, 8, WCOLS], bf16)
        q_sb = P.tile([128, 2, S], bf16)  # [feat(2 heads), pair, token]
        k_sb = P.tile([128, 2, S], bf16)
        v_sb = P.tile([128, 16, 4 * 65], bf16)  # [token, tile, 4*(em + 64 feats)]
        v_blk = v_sb.rearrange("p m (l c) -> p m l c", l=4)
        qkb = P.tile([128, 4], f32)
        qb_sb = qkb[:, 0:2]
        kb_sb = qkb[:, 2:4]
        em_sb = P.tile([128, 16], f32)
        em4_sb = P.tile([128, 64], bf16)

        # ---- input DMAs on two parallel queues (sync + gpsimd; the ACT
        # queue must stay clear for the exp stream). First halves of the
        # hidden-state chunks land first: they cover query/key blocks
        # sp0/sp1, which is all the prologue and the first S steps need.
        nc.sync.dma_start(out=wq_sb[:], in_=wq_r)
        nc.gpsimd.dma_start(out=qb_sb, in_=qb_d.ap())
        nc.gpsimd.dma_start(out=kb_sb, in_=kb_d.ap())
        nc.gpsimd.dma_start(out=wk_sb[:], in_=wk_r)
        h0 = slice(0, 1024)
        h1 = slice(1024, 2048)
        for k in range(4):
            nc.sync.dma_start(out=xt[k][:, h0], in_=xt_r[:, k, h0])
        for k in range(4, 8):
            nc.gpsimd.dma_start(out=xt[k][:, h0], in_=xt_r[:, k, h0])
        nc.gpsimd.dma_start(out=wv_sb[:], in_=wv_r)
        nc.gpsimd.dma_start(out=em_sb[:], in_=em_d.ap())
        nc.gpsimd.dma_start(out=em4_sb[:], in_=em4_d.ap())
        for k in range(4):
            nc.sync.dma_start(out=xt[k][:, h1], in_=xt_r[:, k, h1])
        for k in range(4, 8):
            nc.gpsimd.dma_start(out=xt[k][:, h1], in_=xt_r[:, k, h1])

        # ---- unit generators ----
        # contraction chunks in DMA-arrival order (chunks alternate between
        # the sync and gpsimd queues), so the prologue is never stuck
        # waiting for chunk k while k+1 already landed
        KORD = [0, 4, 1, 5, 2, 6, 3, 7]

        def proj_unit(w_sb, b_sb, dst, mc, sp):
            # one [128 feats, 512 tokens] projection block: 8 matmuls + bias
            pq = PQ.tile([128, 512], f32, tag="pq")
            for i, k in enumerate(KORD):
                nc.tensor.matmul(
                    pq[:],
                    lhsT=w_sb[:, k, mc * 128 : mc * 128 + 128],
                    rhs=xt[k][:, sp * 512 : sp * 512 + 512],
                    start=(i == 0),
                    stop=(i == 7),
                )
            nc.vector.tensor_scalar_add(
                dst[:, mc, sp * 512 : sp * 512 + 512], pq[:], b_sb[:, mc : mc + 1]
            )

        def v_unit(mt):
            # V token tile [128 tokens, 256 feats] + bias row, scaled by exp(mask)
            pv = PQ.tile([128, 512], f32, tag="pq", name="pv")[:, 0:256]
            for i, k in enumerate(KORD):
                nc.tensor.matmul(
                    pv[:],
                    lhsT=xt[k][:, mt * 128 : mt * 128 + 128],
                    rhs=wv_sb[:, k, :],
                    start=(i == 0),
                    stop=(i == 7),
                )
            nc.vector.tensor_scalar_mul(
                v_blk[:, mt, :, 1:65],
                pv.rearrange("p (l c) -> p l c", l=4),
                em_sb[:, mt : mt + 1],
            )
            nc.vector.tensor_copy(v_blk[:, mt, :, 0], em4_sb[:, 4 * mt : 4 * mt + 4])

        def s_kt(p, sp, kt, expP):
            # S^T for key tile kt (both heads of pair p) + mask-free exp
            qs = sp * 512
            ps = PS.tile([128, 1024], f32, tag="ps")
            for h in range(2):
                rs = 64 * h
                nc.tensor.matmul(
                    ps[:, h * 512 : h * 512 + 512],
                    lhsT=k_sb[rs : rs + 64, p, kt * 128 : kt * 128 + 128],
                    rhs=q_sb[rs : rs + 64, p, qs : qs + 512],
                    start=True,
                    stop=True,
                )
            nc.scalar.activation(expP[:, kt, :], ps[:], Exp)

        pcs = {}

        def c_unit(p, sp, half, u, expP):
            # 2 accumulation steps of ctx^T; u==7 finishes with normalize+DMA
            lh = 2 * p + half
            key = (p, sp, half)
            if u == 0:
                pcs[key] = PC.tile([65, 512], f32, tag="pc", name=f"pc{lh}")
            pc = pcs[key]
            for j in range(2):
                kt = 2 * u + j
                nc.tensor.matmul(
                    pc[:],
                    lhsT=v_sb[:, kt, 65 * lh : 65 * lh + 65],
                    rhs=expP[:, kt, half * 512 : half * 512 + 512],
                    start=(kt == 0),
                    stop=(kt == 15),
                )
            if u == 7:
                qs = sp * 512
                ctxs = nrm.tile([65, 512], f32, tag="ctxs")
                bc = nrm.tile([65, 512], f32, tag="bc")
                nq = 2 if (p, sp) == (1, 3) else 1
                for q in range(nq):
                    cs = slice(q * 512 // nq, (q + 1) * 512 // nq)
                    nc.vector.reciprocal_approx_fast(ctxs[0:1, cs], pc[0:1, cs])
                    nc.gpsimd.partition_broadcast(bc[:, cs], ctxs[0:1, cs])
                    nc.vector.tensor_mul(ctxs[:, cs], pc[:, cs], bc[:, cs])
                    nc.sync.dma_start(
                        out=out_d.ap()[
                            64 * lh : 64 * lh + 64, qs + cs.start : qs + cs.stop
                        ],
                        in_=ctxs[1:65, cs],
                    )

        # ---- schedule ----
        # prologue: just Q(0,0) + K(0,s0/s1) -- S(0,0,kt) only needs the
        # key tiles up to kt, so K(0,s2/s3) ride as the first fillers of
        # block (0,0) (they complete before S reaches key tile 8)
        proj_unit(wq_sb, qb_sb, q_sb, 0, 0)
        proj_unit(wk_sb, kb_sb, k_sb, 0, 0)
        proj_unit(wk_sb, kb_sb, k_sb, 0, 1)

        blocks = [(p, sp) for p in (0, 1) for sp in range(4)]
        expPs = {}

        # filler lists per block: context units of the previous block plus
        # projection units needed by upcoming S blocks (emitted late in the
        # block, always before their consumer's S-group in program order).
        def pq_unit(p, sp):
            return ("proj", wq_sb, qb_sb, q_sb, p, sp)

        def pk_unit(p, sp):
            return ("proj", wk_sb, kb_sb, k_sb, p, sp)

        fillers = {
            (0, 0): [("v", mt) for mt in range(8)]
                    + [pk_unit(0, 2), pk_unit(0, 3)]
                    + [("v", mt) for mt in range(8, 16)] + [pq_unit(0, 1)],
            (0, 1): [("c", 0, 0)] + [pq_unit(0, 2)],
            (0, 2): [("c", 0, 1)] + [pq_unit(0, 3)],
            (0, 3): [("c", 0, 2), pk_unit(1, 0), pk_unit(1, 1), pk_unit(1, 2),
                     pk_unit(1, 3), pq_unit(1, 0)],
            (1, 0): [("c", 0, 3), pq_unit(1, 1)],
            (1, 1): [("c", 1, 0), pq_unit(1, 2)],
            (1, 2): [("c", 1, 1), pq_unit(1, 3)],
            (1, 3): [("c", 1, 2)],
        }

        def expand(fill):
            out = []
            for f in fill:
                if f[0] == "c":
                    _, cp, csp = f
                    for u in range(8):
                        for half in range(2):
                            out.append(("cu", cp, csp, half, u))
                else:
                    out.append(f)
            return out

        def emit(unit):
            if unit[0] == "v":
                v_unit(unit[1])
            elif unit[0] == "proj":
                _, w, b, dst, mc, sp = unit
                proj_unit(w, b, dst, mc, sp)
            elif unit[0] == "cu":
                _, cp, csp, half, u = unit
                c_unit(cp, csp, half, u, expPs[(cp, csp)])

        for p, sp in blocks:
            last = (p, sp) == (1, 3)
            expP = att.tile([128, 16, 1024], bf16, tag="expP")
            expPs[(p, sp)] = expP
            fill = expand(fillers[(p, sp)])
            idx = 0
            for kt in range(16):
                s_kt(p, sp, kt, expP)
                if last:
                    # front-load this block's fillers, then chase the exps
                    # with our own context units at a one-key-tile lag
                    budget = 2 if kt < 8 else 0
                    while budget > 0 and idx < len(fill):
                        emit(fill[idx])
                        idx += 1
                        budget -= 1
                    if kt >= 3 and kt % 2 == 1:
                        u = (kt - 3) // 2
                        c_unit(1, 3, 0, u, expP)
                        c_unit(1, 3, 1, u, expP)
                else:
                    want = ((len(fill) - idx) + (15 - kt)) // (16 - kt)
                    for _ in range(want):
                        emit(fill[idx])
                        idx += 1
            while idx < len(fill):
                emit(fill[idx])
                idx += 1

        # epilogue: the last context chunk (its exps just finished)
        for half in range(2):
            c_unit(1, 3, half, 7, expPs[(1, 3)])

    nc.compile()
    return nc


def _get_program():
    if "nc" not in _CACHE:
        _CACHE["nc"] = _build_program()
    return _CACHE["nc"]


def _to_bf16(x):
    import ml_dtypes

    return np.asarray(x, np.float32).astype(ml_dtypes.bfloat16)


def _make_in_maps(hidden_states, attention_mask, q_w, q_b, k_w, k_b, v_w, v_b):
    hs = np.asarray(hidden_states, np.float32)
    am = np.asarray(attention_mask, np.float32)
    q_w = np.asarray(q_w, np.float32)
    k_w = np.asarray(k_w, np.float32)
    v_w = np.asarray(v_w, np.float32)
    q_b = np.asarray(q_b, np.float32)
    k_b = np.asarray(k_b, np.float32)
    v_b = np.asarray(v_b, np.float32)

    scale = np.float32(1.0 / np.sqrt(HD))

    in_maps = []
    for c in range(NCORES):
        b = c // 4
        hg = c % 4
        cols = slice(WCOLS * hg, WCOLS * hg + WCOLS)
        mask = am[b, 0, 0, :]  # [S]
        em = np.exp(mask.reshape(16, 128).T).astype(np.float32)  # [128, 16]
        em4 = np.repeat(em[:, :, None], 4, axis=2).reshape(128, 64)
        in_maps.append(
            {
                "xt": np.ascontiguousarray(_to_bf16(hs[b].T)),
                "wq": np.ascontiguousarray(_to_bf16(q_w[:, cols] * scale)),
                "wk": np.ascontiguousarray(_to_bf16(k_w[:, cols])),
                "wv": np.ascontiguousarray(_to_bf16(v_w[:, cols])),
                "qb2": np.ascontiguousarray((q_b[cols] * scale).reshape(2, 128).T),
                "kb2": np.ascontiguousarray(k_b[cols].reshape(2, 128).T),
                "em": np.ascontiguousarray(em),
                "em4": np.ascontiguousarray(_to_bf16(em4)),
            }
        )
    return in_maps


def kernel(hidden_states, attention_mask, q_w, q_b, k_w, k_b, v_w, v_b):
    from concourse import bass_utils

    nc = _get_program()
    in_maps = _make_in_maps(
        hidden_states, attention_mask, q_w, q_b, k_w, k_b, v_w, v_b
    )
    _CACHE["in_maps"] = in_maps
    res = bass_utils.run_bass_kernel_spmd(nc, in_maps, core_ids=list(range(NCORES)))

    full = np.empty((B, S, HIDDEN), np.float32)
    for c in range(NCORES):
        b = c // 4
        hg = c % 4
        full[b, :, WCOLS * hg : WCOLS * hg + WCOLS] = res.results[c]["out"].T
    # V bias contributes exactly v_b to every context vector (softmax
    # weights sum to 1), so it is added here instead of on-device.
    full += np.asarray(v_b, np.float32)[None, None, :]
    return full
